# revision 24
# baseline (speedup 1.0000x reference)
"""Causal linear attention (elu+1 feature map) on 8 Trainium2 NeuronCores.

Full inputs (n=2, l=2048, h=8, d=64) fp32 are sharded over the 16 (n,h)
head-sequences: core i handles pairs (2i, 2i+1). Each core runs a two-level
chunked scan (chunk C=128, state stride 2 chunks):

  [AT(c) | CROSS] = Kf_c @ [Qf_c | Qf_{c+1}]^T    (one matmul, both pairs)
  AT(c+1)         = Kf_{c+1} @ Qf_{c+1}^T
  out(c)   = ATm(c)^T @ Vaug_c + Qf_c @ S                    ; out /= denom
  out(c+1) = ATm(c+1)^T @ Vaug_{c+1} + CROSS^T @ Vaug_c + Qf_{c+1} @ S
  S       += Kf_c^T @ Vaug_c + Kf_{c+1}^T @ Vaug_{c+1}   (PSUM fp32)

The 2-chunk state stride halves the serial PE->snapshot->PE chain.
Feature map: elu(x)+1 = min(exp(x), max(x+1,1)): exp on ScalarE,
clamp + min on DVE.

qfb layout trick: Q features live in a pair-block structure
qfb[(p',d), 1024p + 128c + i], nonzero only for p'==p (off-blocks zeroed
once; tiles are persistent so the zeros survive). One dense kfT stationary
times this blocked moving operand yields both pairs' AT in one matmul, and
blocked stationaries pull each pair's inter-chunk term from the
(garbage-tolerant) S state. All matmuls keep base-partition-0 operands: PE
quadrant (tile_position) matmuls hang TRN2 when pipelined, as do DVE reads
of the PSUM bank the PE is accumulating S into (the snapshot runs on
ScalarE for that reason).

PSUM accumulation banks get their single start=True from a K=1 all-zeros
matmul; real matmuls all accumulate (start=False) — order-robust, since a
start=True invalidates its whole 2KB PSUM bank.

Host layouts (fp16, all DMAs contiguous):
  qT, kT: (128, 2048)  [(64p + d), (128c + i)]   (host-transposed)
  k,  v : (128, 2048)  [i, 128c + 64p + d]       (natural)
  out   : (128, 2048) fp32, same indexing as k/v.
"""
import numpy as np
from contextlib import ExitStack

import concourse.bacc as bacc
import concourse.bass as bass
import concourse.tile as tile
from concourse import mybir
from concourse.bass_utils import run_bass_kernel_spmd

N, L, H, D = 2, 2048, 8, 64
C = 128                 # chunk length
NCH = L // C            # 16 chunks
GROUP = 8               # chunks per fmap/DMA group
NGRP = NCH // GROUP
PAIRS = 2
W = NCH * PAIRS * D     # 2048
GW = GROUP * PAIRS * D  # 1024 natural cols per group
TW = GROUP * C          # 1024 transposed cols per group
BW = PAIRS * TW         # 2048 blocked cols per group (pair-major)
VW = GROUP * PAIRS * (D + 1)   # 1040 v cols per group (with ones col)
SW = PAIRS * (D + 1)    # 130: S cols [S_p0 | ksum_p0 | S_p1 | ksum_p1]
ATW = 6 * C             # at tile: [ATc p0|CROSS p0|ATc p1|CROSS p1|ATc1 p0p1]

f16 = mybir.dt.float16
f32 = mybir.dt.float32
AF = mybir.ActivationFunctionType
OP = mybir.AluOpType


def _fmap(nc, pool, src, width, tag):
    """f = min(exp(x), max(x+1,1)): exp on ACT, clamp + min on DVE."""
    e = pool.tile([C, width], f16, tag=f"e_{tag}")
    t = pool.tile([C, width], f16, tag=f"t_{tag}")
    nc.scalar.activation(e, src, AF.Exp)
    nc.vector.tensor_scalar(out=t, in0=src, scalar1=1.0, scalar2=1.0,
                            op0=OP.add, op1=OP.max)
    return e, t


def build_kernel():
    nc = bacc.Bacc("TRN2", target_bir_lowering=False, debug=False, num_devices=8)
    qT_d = nc.dram_tensor("qT", (C, W), f16, kind="ExternalInput").ap()
    kT_d = nc.dram_tensor("kT", (C, W), f16, kind="ExternalInput").ap()
    k_d = nc.dram_tensor("k", (C, W), f16, kind="ExternalInput").ap()
    v_d = nc.dram_tensor("v", (C, W), f16, kind="ExternalInput").ap()
    o_d = nc.dram_tensor("o", (C, W), f32, kind="ExternalOutput").ap()

    with tile.TileContext(nc) as tc, ExitStack() as ctx:
        consts = ctx.enter_context(tc.tile_pool(name="consts", bufs=1))
        io_pool = ctx.enter_context(tc.tile_pool(name="io", bufs=2))
        fm_pool = ctx.enter_context(tc.tile_pool(name="fm", bufs=2))
        sm_pool = ctx.enter_context(tc.tile_pool(name="sm", bufs=3))
        at_psum = ctx.enter_context(tc.tile_pool(name="at", bufs=2, space="PSUM"))
        out_psum = ctx.enter_context(tc.tile_pool(name="out", bufs=3, space="PSUM"))
        s_psum = ctx.enter_context(tc.tile_pool(name="sp", bufs=1, space="PSUM"))

        zeros = consts.tile([1, 2 * SW], f16)
        nc.vector.memset(zeros, 0.0)

        # mask blocks: [tri, ones, tri, ones, tri, tri] (128 cols each)
        maskT = consts.tile([C, ATW], f32)
        m6 = maskT.rearrange("j (b i) -> j b i", b=6)
        nc.gpsimd.memset(maskT, 0.0)
        for blk in ((0, 1), (2, 3), (4, 6)):
            nc.gpsimd.affine_select(
                out=m6[:, blk[0]:blk[1]], in_=m6[:, blk[0]:blk[1]],
                compare_op=OP.is_gt, fill=1.0,
                base=0, pattern=[[0, blk[1] - blk[0]], [-1, C]],
                channel_multiplier=1,
            )
        nc.gpsimd.memset(m6[:, 1:2], 1.0)
        nc.gpsimd.memset(m6[:, 3:4], 1.0)

        # persistent running state (off-pair blocks accumulate unread garbage)
        S_ps = s_psum.tile([C, SW], f32)
        nc.tensor.matmul(S_ps, zeros[:, 0:C], zeros[:, 0:SW],
                         start=True, stop=False, skip_group_check=True)

        # persistent double-buffered tiles: qfb off-blocks and the v ones
        # columns are written once and never touched by per-group writes
        qfbs, vgs = [], []
        for b in range(2):
            qfb = consts.tile([C, BW], f16, tag=f"qfb{b}")
            nc.vector.memset(qfb, 0.0)
            qfbs.append(qfb)
            v_g = consts.tile([C, VW], f16, tag=f"v_g{b}")
            v4 = v_g.rearrange("i (j b x) -> i j b x", j=GROUP, b=PAIRS)
            nc.vector.memset(v4[:, :, :, D:D + 1], 1.0)
            vgs.append(v_g)

        for g in range(NGRP):
            gsl = slice(g * GW, (g + 1) * GW)
            tsl = slice(g * TW, (g + 1) * TW)

            qT_g = io_pool.tile([C, TW], f16, tag="qT_g")
            kT_g = io_pool.tile([C, TW], f16, tag="kT_g")
            k_g = io_pool.tile([C, GW], f16, tag="k_g")
            v_g = vgs[g % 2]
            v4 = v_g.rearrange("i (j b x) -> i j b x", j=GROUP, b=PAIRS)
            nc.sync.dma_start(qT_g, qT_d[:, tsl])
            nc.sync.dma_start(kT_g, kT_d[:, tsl])
            nc.sync.dma_start(k_g, k_d[:, gsl])
            nc.sync.dma_start(
                v4[:, :, :, 0:D],
                v_d[:, gsl].rearrange("i (j b x) -> i j b x", j=GROUP, b=PAIRS),
            )

            # feature maps
            e_q, t_q = _fmap(nc, fm_pool, qT_g, TW, "q")
            qfb = qfbs[g % 2]
            for p in range(PAIRS):
                rows = slice(p * D, (p + 1) * D)
                nc.vector.tensor_tensor(
                    out=qfb[rows, p * TW:(p + 1) * TW],
                    in0=e_q[rows], in1=t_q[rows], op=OP.min)
            qfb3 = qfb.rearrange("r (p x) -> r p x", p=PAIRS)

            e_kT, t_kT = _fmap(nc, fm_pool, kT_g, TW, "kT")
            kfT = fm_pool.tile([C, TW], f16, tag="kfT")
            nc.vector.tensor_tensor(out=kfT, in0=e_kT, in1=t_kT, op=OP.min)

            e_k, t_k = _fmap(nc, fm_pool, k_g, GW, "k")
            kf = fm_pool.tile([C, GW], f16, tag="kf")
            nc.vector.tensor_tensor(out=kf, in0=e_k, in1=t_k, op=OP.min)

            stage = io_pool.tile([C, GW], f32, tag="stage")

            for jj in range(GROUP // 2):    # two chunks per scan step
                j0, j1 = 2 * jj, 2 * jj + 1
                c0 = g * GROUP + j0
                t0 = slice(j0 * C, (j0 + 1) * C)
                t1 = slice(j1 * C, (j1 + 1) * C)
                t01 = slice(j0 * C, (j1 + 1) * C)

                at_ps = at_psum.tile([C, ATW], f32, tag="at")
                # [AT(c) | CROSS] both pairs in one matmul (shared stationary)
                nc.tensor.matmul(at_ps[:, 0:4 * C], kfT[:, t0],
                                 qfb3[:, :, t01], start=True, stop=True)
                nc.tensor.matmul(at_ps[:, 4 * C:6 * C], kfT[:, t1],
                                 qfb3[:, :, t1], start=True, stop=True)

                out_ps = out_psum.tile([C, 2 * SW], f32, tag="out")
                nc.tensor.matmul(out_ps, zeros[:, 0:C], zeros[:, 0:2 * SW],
                                 start=True, stop=False, skip_group_check=True)

                # state snapshot (state through chunk c0-1); ScalarE on purpose
                if c0 > 0:
                    S_sb = sm_pool.tile([C, SW], f16, tag="s_sb")
                    nc.scalar.copy(S_sb, S_ps)
                    for dj, tx in ((0, t0), (1, t1)):
                        for p in range(PAIRS):
                            vs = slice(p * (D + 1), (p + 1) * (D + 1))
                            nc.tensor.matmul(
                                out_ps[:, dj * SW + vs.start:dj * SW + vs.stop],
                                qfb[:, p * TW + tx.start:p * TW + tx.stop],
                                S_sb[:, vs],
                                start=False, stop=False, skip_group_check=True)

                # state updates, both chunks (after the snapshot read)
                for j, c in ((j0, c0), (j1, c0 + 1)):
                    if c < NCH - 1:
                        nc.tensor.matmul(
                            S_ps,
                            kf[:, j * PAIRS * D:(j + 1) * PAIRS * D],
                            v_g[:, j * SW:(j + 1) * SW],
                            start=False, stop=(c == NCH - 2),
                            skip_group_check=True)

                # mask ATs + copy CROSS in one DVE op
                atm = sm_pool.tile([C, ATW], f16, tag="atm")
                nc.vector.tensor_mul(atm, at_ps, maskT)

                # intra-chunk + cross contributions
                for p in range(PAIRS):
                    vs0 = slice(p * (D + 1), (p + 1) * (D + 1))
                    nc.tensor.matmul(        # out1(c0)
                        out_ps[:, vs0],
                        atm[:, 2 * p * C:(2 * p + 1) * C], v4[:, j0, p, :],
                        start=False, stop=False, skip_group_check=True)
                    nc.tensor.matmul(        # cross -> c1
                        out_ps[:, SW + vs0.start:SW + vs0.stop],
                        atm[:, (2 * p + 1) * C:(2 * p + 2) * C], v4[:, j0, p, :],
                        start=False, stop=False, skip_group_check=True)
                    nc.tensor.matmul(        # out1(c1)
                        out_ps[:, SW + vs0.start:SW + vs0.stop],
                        atm[:, (4 + p) * C:(5 + p) * C], v4[:, j1, p, :],
                        start=False, stop=(p == PAIRS - 1),
                        skip_group_check=True)

                # out = num * (1/den) for both chunks+pairs
                o5 = out_ps.rearrange("i (a b x) -> i a b x", a=2, b=PAIRS)
                recip = sm_pool.tile([C, 2, PAIRS, 1], f32, tag="recip")
                nc.vector.reciprocal(recip, o5[:, :, :, D:D + 1])
                rec_b = bass.AP(
                    tensor=recip.tensor, offset=recip.offset,
                    ap=[list(recip.ap[0]), list(recip.ap[1]),
                        list(recip.ap[2]), [0, D]],
                )
                st4 = stage.rearrange(
                    "i (j b x) -> i j b x", j=GROUP, b=PAIRS)[:, 2 * jj:2 * jj + 2]
                nc.vector.tensor_tensor(
                    out=st4, in0=o5[:, :, :, 0:D], in1=rec_b, op=OP.mult)

            nc.sync.dma_start(o_d[:, gsl], stage)

    nc.compile()
    return nc


_nc_cache = None


def _get_nc():
    global _nc_cache
    if _nc_cache is None:
        _nc_cache = build_kernel()
    return _nc_cache


def _core_pairs(x, core):
    flat = x.transpose(0, 2, 1, 3).reshape(N * H, L, D)
    return flat[2 * core:2 * core + 2]          # (2, L, D) fp32


def _nat_layout(xc):
    # (2, L, D) -> (128, 2048) [i, 128c + 64p + d]
    return np.ascontiguousarray(
        xc.reshape(PAIRS, NCH, C, D).transpose(2, 1, 0, 3).reshape(C, W)
    ).astype(np.float16)


def _t_layout(xc):
    # (2, L, D) -> (128, 2048) [(64p + d), (128c + i)]
    return np.ascontiguousarray(
        xc.reshape(PAIRS, NCH, C, D).transpose(0, 3, 1, 2).reshape(C, W)
    ).astype(np.float16)


def make_in_maps(queries, keys, values):
    in_maps = []
    for core in range(8):
        qc = _core_pairs(queries, core)
        kc = _core_pairs(keys, core)
        vc = _core_pairs(values, core)
        in_maps.append({
            "qT": _t_layout(qc),
            "kT": _t_layout(kc),
            "k": _nat_layout(kc),
            "v": _nat_layout(vc),
        })
    return in_maps


def kernel(queries, keys, values):
    nc = _get_nc()
    in_maps = make_in_maps(queries, keys, values)
    res = run_bass_kernel_spmd(nc, in_maps, core_ids=list(range(8)))
    out = np.zeros((N, L, H, D), np.float32)
    for core in range(8):
        oc = res.results[core]["o"].reshape(C, NCH, PAIRS, D)
        oc = oc.transpose(2, 1, 0, 3).reshape(PAIRS, L, D)
        for p in range(PAIRS):
            flat = 2 * core + p
            out[flat // H, :, flat % H, :] = oc[p]
    return out


# revision 29
# speedup vs baseline: 1.0630x; 1.0630x over previous
"""Causal linear attention (elu+1 feature map) on 8 Trainium2 NeuronCores.

Full inputs (n=2, l=2048, h=8, d=64) fp32 are sharded over the 16 (n,h)
head-sequences: core i handles pairs (2i, 2i+1). Each core runs a two-level
chunked scan (chunk C=128, state stride 2 chunks):

  [AT(c) | CROSS] = Kf_c @ [Qf_c | Qf_{c+1}]^T    (one matmul, both pairs)
  AT(c+1)         = Kf_{c+1} @ Qf_{c+1}^T
  out(c)   = ATm(c)^T @ Vaug_c + Qf_c @ S                    ; out /= denom
  out(c+1) = ATm(c+1)^T @ Vaug_{c+1} + CROSS^T @ Vaug_c + Qf_{c+1} @ S
  S       += Kf_c^T @ Vaug_c + Kf_{c+1}^T @ Vaug_{c+1}   (PSUM fp32)

The 2-chunk state stride halves the serial PE->snapshot->PE chain.
Feature map: elu(x)+1 = min(exp(x), max(x+1,1)): exp on ScalarE,
clamp + min on DVE.

qfb layout trick: Q features live in a pair-block structure
qfb[(p',d), 1024p + 128c + i], nonzero only for p'==p (off-blocks zeroed
once; tiles are persistent so the zeros survive). One dense kfT stationary
times this blocked moving operand yields both pairs' AT in one matmul, and
blocked stationaries pull each pair's inter-chunk term from the
(garbage-tolerant) S state. All matmuls keep base-partition-0 operands: PE
quadrant (tile_position) matmuls hang TRN2 when pipelined, as do DVE reads
of the PSUM bank the PE is accumulating S into (the snapshot runs on
ScalarE for that reason).

PSUM accumulation banks get their single start=True from a K=1 all-zeros
matmul; real matmuls all accumulate (start=False) — order-robust, since a
start=True invalidates its whole 2KB PSUM bank.

Host layouts (fp16, all DMAs contiguous):
  qT, kT: (128, 2048)  [(64p + d), (128c + i)]   (host-transposed)
  k,  v : (128, 2048)  [i, 128c + 64p + d]       (natural)
  out   : (128, 2048) fp32, same indexing as k/v.
"""
import numpy as np
from contextlib import ExitStack

import concourse.bacc as bacc
import concourse.bass as bass
import concourse.tile as tile
from concourse import mybir
from concourse.bass_utils import run_bass_kernel_spmd

N, L, H, D = 2, 2048, 8, 64
C = 128                 # chunk length
NCH = L // C            # 16 chunks
GROUP = 8               # chunks per fmap/DMA group
NGRP = NCH // GROUP
PAIRS = 2
W = NCH * PAIRS * D     # 2048
GW = GROUP * PAIRS * D  # 1024 natural cols per group
TW = GROUP * C          # 1024 transposed cols per group
BW = PAIRS * TW         # 2048 blocked cols per group (pair-major)
VW = GROUP * PAIRS * (D + 1)   # 1040 v cols per group (with ones col)
SW = PAIRS * (D + 1)    # 130: S cols [S_p0 | ksum_p0 | S_p1 | ksum_p1]
ATW = 6 * C             # at tile: [ATc p0|CROSS p0|ATc p1|CROSS p1|ATc1 p0p1]

f16 = mybir.dt.float16
f32 = mybir.dt.float32
AF = mybir.ActivationFunctionType
OP = mybir.AluOpType


def _fmap(nc, pool, src, width, tag):
    """f = min(exp(x), max(x+1,1)): exp on ACT, clamp + min on DVE."""
    e = pool.tile([C, width], f16, tag=f"e_{tag}")
    t = pool.tile([C, width], f16, tag=f"t_{tag}")
    nc.scalar.activation(e, src, AF.Exp)
    nc.vector.tensor_scalar(out=t, in0=src, scalar1=1.0, scalar2=1.0,
                            op0=OP.add, op1=OP.max)
    return e, t


def build_kernel():
    nc = bacc.Bacc("TRN2", target_bir_lowering=False, debug=False, num_devices=8)
    qT_d = nc.dram_tensor("qT", (C, W), f16, kind="ExternalInput").ap()
    kT_d = nc.dram_tensor("kT", (C, W), f16, kind="ExternalInput").ap()
    k_d = nc.dram_tensor("k", (C, W), f16, kind="ExternalInput").ap()
    v_d = nc.dram_tensor("v", (C, W), f16, kind="ExternalInput").ap()
    o_d = nc.dram_tensor("o", (C, W), f32, kind="ExternalOutput").ap()

    with tile.TileContext(nc) as tc, ExitStack() as ctx:
        consts = ctx.enter_context(tc.tile_pool(name="consts", bufs=1))
        io_pool = ctx.enter_context(tc.tile_pool(name="io", bufs=2))
        fm_pool = ctx.enter_context(tc.tile_pool(name="fm", bufs=2))
        sm_pool = ctx.enter_context(tc.tile_pool(name="sm", bufs=3))
        at_psum = ctx.enter_context(tc.tile_pool(name="at", bufs=2, space="PSUM"))
        out_psum = ctx.enter_context(tc.tile_pool(name="out", bufs=3, space="PSUM"))
        s_psum = ctx.enter_context(tc.tile_pool(name="sp", bufs=1, space="PSUM"))

        zeros = consts.tile([1, 4 * C], f16)
        nc.vector.memset(zeros, 0.0)

        # PE warm-up: ~3.4us of dep-free matmul activity during the initial
        # DMA wait flips the HAM clock gate to 2.4GHz before real work lands.
        for _ in range(7):
            warm = at_psum.tile([C, 4 * C], f32, tag="at")
            nc.tensor.matmul(warm, zeros[:, 0:C], zeros[:, 0:4 * C],
                             start=True, stop=True)

        # mask blocks: [tri, ones, tri, ones, tri, tri] (128 cols each)
        maskT = consts.tile([C, ATW], f32)
        m6 = maskT.rearrange("j (b i) -> j b i", b=6)
        nc.gpsimd.memset(maskT, 0.0)
        for blk in ((0, 1), (2, 3), (4, 6)):
            nc.gpsimd.affine_select(
                out=m6[:, blk[0]:blk[1]], in_=m6[:, blk[0]:blk[1]],
                compare_op=OP.is_gt, fill=1.0,
                base=0, pattern=[[0, blk[1] - blk[0]], [-1, C]],
                channel_multiplier=1,
            )
        nc.gpsimd.memset(m6[:, 1:2], 1.0)
        nc.gpsimd.memset(m6[:, 3:4], 1.0)

        # persistent running state (off-pair blocks accumulate unread garbage)
        S_ps = s_psum.tile([C, SW], f32)
        nc.tensor.matmul(S_ps, zeros[:, 0:C], zeros[:, 0:SW],
                         start=True, stop=False, skip_group_check=True)

        # persistent double-buffered tiles: qfb off-blocks and the v ones
        # columns are written once and never touched by per-group writes
        qfbs, vgs = [], []
        for b in range(2):
            qfb = consts.tile([C, BW], f16, tag=f"qfb{b}")
            nc.vector.memset(qfb, 0.0)
            qfbs.append(qfb)
            v_g = consts.tile([C, VW], f16, tag=f"v_g{b}")
            v4 = v_g.rearrange("i (j b x) -> i j b x", j=GROUP, b=PAIRS)
            nc.vector.memset(v4[:, :, :, D:D + 1], 1.0)
            vgs.append(v_g)

        for g in range(NGRP):
            gsl = slice(g * GW, (g + 1) * GW)
            tsl = slice(g * TW, (g + 1) * TW)

            qT_g = io_pool.tile([C, TW], f16, tag="qT_g")
            kT_g = io_pool.tile([C, TW], f16, tag="kT_g")
            k_g = io_pool.tile([C, GW], f16, tag="k_g")
            v_g = vgs[g % 2]
            v4 = v_g.rearrange("i (j b x) -> i j b x", j=GROUP, b=PAIRS)
            nc.sync.dma_start(qT_g, qT_d[:, tsl])
            nc.sync.dma_start(kT_g, kT_d[:, tsl])
            nc.sync.dma_start(k_g, k_d[:, gsl])
            nc.sync.dma_start(
                v4[:, :, :, 0:D],
                v_d[:, gsl].rearrange("i (j b x) -> i j b x", j=GROUP, b=PAIRS),
            )

            # feature maps
            e_q, t_q = _fmap(nc, fm_pool, qT_g, TW, "q")
            qfb = qfbs[g % 2]
            for p in range(PAIRS):
                rows = slice(p * D, (p + 1) * D)
                nc.vector.tensor_tensor(
                    out=qfb[rows, p * TW:(p + 1) * TW],
                    in0=e_q[rows], in1=t_q[rows], op=OP.min)
            qfb3 = qfb.rearrange("r (p x) -> r p x", p=PAIRS)

            e_kT, t_kT = _fmap(nc, fm_pool, kT_g, TW, "kT")
            kfT = fm_pool.tile([C, TW], f16, tag="kfT")
            nc.vector.tensor_tensor(out=kfT, in0=e_kT, in1=t_kT, op=OP.min)

            e_k, t_k = _fmap(nc, fm_pool, k_g, GW, "k")
            kf = fm_pool.tile([C, GW], f16, tag="kf")
            nc.vector.tensor_tensor(out=kf, in0=e_k, in1=t_k, op=OP.min)

            stage = io_pool.tile([C, GW], f32, tag="stage")

            for jj in range(GROUP // 2):    # two chunks per scan step
                j0, j1 = 2 * jj, 2 * jj + 1
                c0 = g * GROUP + j0
                t0 = slice(j0 * C, (j0 + 1) * C)
                t1 = slice(j1 * C, (j1 + 1) * C)
                t01 = slice(j0 * C, (j1 + 1) * C)

                at_ps = at_psum.tile([C, ATW], f32, tag="at")
                # [AT(c) | CROSS] both pairs in one matmul (shared stationary)
                nc.tensor.matmul(at_ps[:, 0:4 * C], kfT[:, t0],
                                 qfb3[:, :, t01], start=True, stop=True)
                nc.tensor.matmul(at_ps[:, 4 * C:6 * C], kfT[:, t1],
                                 qfb3[:, :, t1], start=True, stop=True)

                out_ps = out_psum.tile([C, 2 * SW], f32, tag="out")
                nc.tensor.matmul(out_ps, zeros[:, 0:C], zeros[:, 0:2 * SW],
                                 start=True, stop=False, skip_group_check=True)

                # state snapshot (state through chunk c0-1); ScalarE on purpose
                if c0 > 0:
                    S_sb = sm_pool.tile([C, SW], f16, tag="s_sb")
                    nc.scalar.copy(S_sb, S_ps)
                    for dj, tx in ((0, t0), (1, t1)):
                        for p in range(PAIRS):
                            vs = slice(p * (D + 1), (p + 1) * (D + 1))
                            nc.tensor.matmul(
                                out_ps[:, dj * SW + vs.start:dj * SW + vs.stop],
                                qfb[:, p * TW + tx.start:p * TW + tx.stop],
                                S_sb[:, vs],
                                start=False, stop=False, skip_group_check=True)

                # state updates, both chunks (after the snapshot read)
                for j, c in ((j0, c0), (j1, c0 + 1)):
                    if c < NCH - 1:
                        nc.tensor.matmul(
                            S_ps,
                            kf[:, j * PAIRS * D:(j + 1) * PAIRS * D],
                            v_g[:, j * SW:(j + 1) * SW],
                            start=False, stop=(c == NCH - 2),
                            skip_group_check=True)

                # mask ATs + copy CROSS in one DVE op
                atm = sm_pool.tile([C, ATW], f16, tag="atm")
                nc.vector.tensor_mul(atm, at_ps, maskT)

                # intra-chunk + cross contributions
                for p in range(PAIRS):
                    vs0 = slice(p * (D + 1), (p + 1) * (D + 1))
                    nc.tensor.matmul(        # out1(c0)
                        out_ps[:, vs0],
                        atm[:, 2 * p * C:(2 * p + 1) * C], v4[:, j0, p, :],
                        start=False, stop=False, skip_group_check=True)
                    nc.tensor.matmul(        # cross -> c1
                        out_ps[:, SW + vs0.start:SW + vs0.stop],
                        atm[:, (2 * p + 1) * C:(2 * p + 2) * C], v4[:, j0, p, :],
                        start=False, stop=False, skip_group_check=True)
                    nc.tensor.matmul(        # out1(c1)
                        out_ps[:, SW + vs0.start:SW + vs0.stop],
                        atm[:, (4 + p) * C:(5 + p) * C], v4[:, j1, p, :],
                        start=False, stop=(p == PAIRS - 1),
                        skip_group_check=True)

                # out = num * (1/den) for both chunks+pairs
                o5 = out_ps.rearrange("i (a b x) -> i a b x", a=2, b=PAIRS)
                recip = sm_pool.tile([C, 2, PAIRS, 1], f32, tag="recip")
                nc.vector.reciprocal(recip, o5[:, :, :, D:D + 1])
                rec_b = bass.AP(
                    tensor=recip.tensor, offset=recip.offset,
                    ap=[list(recip.ap[0]), list(recip.ap[1]),
                        list(recip.ap[2]), [0, D]],
                )
                st4 = stage.rearrange(
                    "i (j b x) -> i j b x", j=GROUP, b=PAIRS)[:, 2 * jj:2 * jj + 2]
                nc.vector.tensor_tensor(
                    out=st4, in0=o5[:, :, :, 0:D], in1=rec_b, op=OP.mult)

            nc.sync.dma_start(o_d[:, gsl], stage)

    nc.compile()
    return nc


_nc_cache = None


def _get_nc():
    global _nc_cache
    if _nc_cache is None:
        _nc_cache = build_kernel()
    return _nc_cache


def _core_pairs(x, core):
    flat = x.transpose(0, 2, 1, 3).reshape(N * H, L, D)
    return flat[2 * core:2 * core + 2]          # (2, L, D) fp32


def _nat_layout(xc):
    # (2, L, D) -> (128, 2048) [i, 128c + 64p + d]
    return np.ascontiguousarray(
        xc.reshape(PAIRS, NCH, C, D).transpose(2, 1, 0, 3).reshape(C, W)
    ).astype(np.float16)


def _t_layout(xc):
    # (2, L, D) -> (128, 2048) [(64p + d), (128c + i)]
    return np.ascontiguousarray(
        xc.reshape(PAIRS, NCH, C, D).transpose(0, 3, 1, 2).reshape(C, W)
    ).astype(np.float16)


def make_in_maps(queries, keys, values):
    in_maps = []
    for core in range(8):
        qc = _core_pairs(queries, core)
        kc = _core_pairs(keys, core)
        vc = _core_pairs(values, core)
        in_maps.append({
            "qT": _t_layout(qc),
            "kT": _t_layout(kc),
            "k": _nat_layout(kc),
            "v": _nat_layout(vc),
        })
    return in_maps


def kernel(queries, keys, values):
    nc = _get_nc()
    in_maps = make_in_maps(queries, keys, values)
    res = run_bass_kernel_spmd(nc, in_maps, core_ids=list(range(8)))
    out = np.zeros((N, L, H, D), np.float32)
    for core in range(8):
        oc = res.results[core]["o"].reshape(C, NCH, PAIRS, D)
        oc = oc.transpose(2, 1, 0, 3).reshape(PAIRS, L, D)
        for p in range(PAIRS):
            flat = 2 * core + p
            out[flat // H, :, flat % H, :] = oc[p]
    return out
